# revision 11
# baseline (speedup 1.0000x reference)
"""Two-layer GCN (BotGCN) on 8 Trainium2 NeuronCores.

Distribution: nodes partitioned contiguously across the 8 cores (12500
each). Each core owns the edges whose destination lands in its block.
Layer math is refactored so all per-edge work is a gather of pre-scaled
rows + a segment-sum:

    out[v] = dinv[v] * (sum_{e: dst=v, real} (dinv[src] * h[src])
                        + dinv[v] * h[v]) + bias

Self-loops are NOT materialized as edges: their contribution is the
elementwise term dinv[v]^2 * h[v], added from an SBUF-resident copy of
the local pre-scaled rows. This removes ~12.5K slots/core/layer and the
per-bucket self-loop concentration that inflated padding.

Per layer, per core:
  - compute locally-owned pre-scaled rows in bf16, AllGather them into a
    [8*12544, 128] bf16 table (row = 64 feats + 64 junk pad columns that
    are never read; 256B rows satisfy the gather's elem-size rule)
  - dma_gather edge messages from the table (4 windows of 2 shards so
    int16 gather indices stay in range); gather chunks rotate across the
    4 SWDGE queues so Q7 descriptor generation runs on all 4 core pairs
    concurrently (~4x)
  - segment-sum via bf16 iota-compare one-hot matrices and single-pass
    bf16 PE matmuls accumulating into fp32 PSUM, one [128, 64] tile per
    128-destination block
Layer 2 then applies W2 on the aggregate (linearity) and scales.

The edge bucketing / padding structure is computed on the host from the
integer edge list and baked into the program (the program is rebuilt per
kernel() call); it is shared by all 8 cores (max-over-cores group
counts) because every core runs the same NEFF.
"""

import numpy as np

N = 100000
NCORES = 8
NPC = N // NCORES            # 12500 nodes per core
SHARD = 12544                # 128*98: 12500 rows + zero row @12500 + pad
WIN = 2 * SHARD              # 25088 table rows per gather window (< 32768)
NWIN = 4
BLK = 128
NBLK = (NPC + BLK - 1) // BLK          # 98 destination blocks
LAST_BLK = NPC - (NBLK - 1) * BLK      # 84 nodes in the last block
F_IN, F_HID, F_OUT = 128, 64, 2
TBLW = 128                              # bf16 table row width (256B)
ZROW = NPC                              # zero row offset within a shard
# 3 passes of <=33 dst blocks: each pass holds ceil(33/8)=5 PSUM bank tiles,
# leaving banks for the transient pool (PSUM tiles pad to a full bank).
PASSES = [list(range(0, 33)), list(range(33, 66)), list(range(66, NBLK))]
CHUNK_GROUPS = 32                                      # 4096 slots per gather

_CACHE = {}

# Results of the most recent run (for the local test harness's profiling).
LAST_RESULTS = None


def _preprocess(edge_index):
    """Host-side integer bucketing of the edge list (self-loops excluded).

    Returns the shared group structure G[b, w], slot count S, and the
    per-core staged gather-index / dstrel arrays.
    """
    src = np.asarray(edge_index[0]).astype(np.int64)
    dst = np.asarray(edge_index[1]).astype(np.int64)

    # degree includes the self-loop (reference semantics)
    deg = (np.bincount(dst, minlength=N) + 1).astype(np.float32)

    core = dst // NPC
    dloc = dst % NPC
    blk = dloc // BLK
    win = src // (2 * NPC)

    cnt = np.zeros((NCORES, NBLK, NWIN), np.int64)
    np.add.at(cnt, (core, blk, win), 1)
    G = -(-cnt.max(axis=0) // BLK)        # [NBLK, NWIN] ceil, shared by cores

    # Bucket ordering: (pass, window, block) — must match the kernel loops.
    bucket_order = []
    for p in range(len(PASSES)):
        for w in range(NWIN):
            for b in PASSES[p]:
                bucket_order.append((b, w))
    nbuckets = len(bucket_order)
    ord_of = np.zeros((NBLK, NWIN), np.int64)
    sizes = np.zeros(nbuckets, np.int64)
    for i, (b, w) in enumerate(bucket_order):
        ord_of[b, w] = i
        sizes[i] = G[b, w] * BLK
    offs = np.zeros(nbuckets + 1, np.int64)
    np.cumsum(sizes, out=offs[1:])
    S = int(offs[-1])

    starts = offs[:-1]

    per_core = []
    for c in range(NCORES):
        m = core == c
        sc = src[m]
        key = ord_of[blk[m], win[m]]
        order = np.argsort(key, kind="stable")
        ks = key[order]
        bstart = np.searchsorted(ks, np.arange(nbuckets))
        rank = np.arange(len(ks)) - bstart[ks]
        slot = starts[ks] + rank

        so = sc[order]
        o = so // NPC
        wrel = (o % 2) * SHARD + (so % NPC)

        gidx = np.full(S, ZROW, np.int16)         # pad -> window zero row
        gidx[slot] = wrel.astype(np.int16)
        drel = np.full(S, -1.0, np.float32)
        drel[slot] = (dloc[m][order] % BLK).astype(np.float32)

        gidx16 = gidx.reshape(S // 16, 16).T      # [16, S/16], slot i -> [i%16, i//16]
        gidx_rep = np.tile(gidx16, (8, 1)).copy() # replicated for the 8 Q7 cores
        import ml_dtypes
        # precomputed one-hot matrices, lhsT layout: ohm[p, g*128+c] =
        # (drel[g*128+p] == c); pads (-1) give all-zero rows
        og = drel.reshape(S // BLK, BLK)
        ohm = (og[:, :, None] == np.arange(BLK, dtype=np.float32)[None, None, :])
        ohm = ohm.transpose(1, 0, 2).reshape(BLK, S).astype(ml_dtypes.bfloat16)

        degc = np.ones(NBLK * BLK, np.float32)
        degc[:NPC] = deg[c * NPC:(c + 1) * NPC]
        degT = degc.reshape(NBLK, BLK).T.copy()   # [128, NBLK]

        per_core.append({"gidx": gidx_rep, "ohm": ohm, "degT": degT})

    return G, S, per_core


def _build(G, S, b1_nonzero, b2_nonzero):
    import concourse.bacc as bacc
    import concourse.mybir as mybir
    import concourse.tile as tile
    from concourse.masks import make_identity

    f32 = mybir.dt.float32
    bf16 = mybir.dt.bfloat16
    AT = mybir.AluOpType

    # first/last matmul (w, b, g) per PSUM bank for start/stop flags.
    first, last = {}, {}
    for p in range(len(PASSES)):
        for w in range(NWIN):
            for b in PASSES[p]:
                bank = PASSES[p].index(b) // 8
                for g in range(int(G[b, w])):
                    last[(p, bank)] = (w, b, g)
                    first.setdefault((p, bank), (w, b, g))

    nc = bacc.Bacc("TRN2", target_bir_lowering=False, debug=False,
                   enable_asserts=True, num_devices=NCORES,
                   num_swdge_queues=4)
    xT = nc.dram_tensor("xT", [F_IN, NPC], bf16, kind="ExternalInput")
    W1 = nc.dram_tensor("W1", [F_IN, F_HID], bf16, kind="ExternalInput")
    W2 = nc.dram_tensor("W2", [F_HID, F_OUT], bf16, kind="ExternalInput")
    b1r = nc.dram_tensor("b1r", [BLK, F_HID], f32, kind="ExternalInput")
    b2r = nc.dram_tensor("b2r", [BLK, F_OUT], f32, kind="ExternalInput")
    degT = nc.dram_tensor("degT", [BLK, NBLK], f32, kind="ExternalInput")
    gidx = nc.dram_tensor("gidx", [BLK, S // 16], mybir.dt.int16,
                          kind="ExternalInput")
    ohm = nc.dram_tensor("ohm", [BLK, S], bf16, kind="ExternalInput")
    y = nc.dram_tensor("y", [NPC, F_OUT], f32, kind="ExternalOutput")

    with tile.TileContext(nc) as tc:
        with tc.tile_pool(name="const", bufs=1) as const, \
             tc.tile_pool(name="xt", bufs=3) as xpool, \
             tc.tile_pool(name="hs", bufs=3) as hpool, \
             tc.tile_pool(name="msgs", bufs=8) as mpool, \
             tc.tile_pool(name="oh", bufs=8) as ohpool, \
             tc.tile_pool(name="post", bufs=3) as ppool, \
             tc.tile_pool(name="psb", bufs=1, space="PSUM") as psb, \
             tc.tile_pool(name="pst", bufs=2, space="PSUM") as pst, \
             tc.tile_pool(name="dram", bufs=1, space="DRAM") as dram:

            ag1_in = dram.tile([SHARD, TBLW], bf16)
            ag1_out = dram.tile([NCORES * SHARD, TBLW], bf16,
                                addr_space="Shared")
            ag2_in = dram.tile([SHARD, TBLW], bf16)
            ag2_out = dram.tile([NCORES * SHARD, TBLW], bf16,
                                addr_space="Shared")

            # ---- constants ----
            ident = const.tile([BLK, BLK], f32)
            make_identity(nc, ident[:])
            W1t = const.tile([F_IN, F_HID], bf16)
            nc.sync.dma_start(W1t[:], W1[:])
            W2t = const.tile([F_HID, F_OUT], bf16)
            nc.sync.dma_start(W2t[:], W2[:])
            if b1_nonzero:
                b1t = const.tile([BLK, F_HID], f32)
                nc.sync.dma_start(b1t[:], b1r[:])
            if b2_nonzero:
                b2t = const.tile([BLK, F_OUT], f32)
                nc.sync.dma_start(b2t[:], b2r[:])
            degt = const.tile([BLK, NBLK], f32)
            nc.sync.dma_start(degt[:], degT[:])
            rcp = const.tile([BLK, NBLK], f32)
            nc.vector.reciprocal(rcp[:], degt[:])
            dinv = const.tile([BLK, NBLK], f32)
            nc.scalar.sqrt(dinv[:], rcp[:])
            dinv2 = const.tile([BLK, NBLK], f32)
            nc.vector.tensor_mul(dinv2[:], dinv[:], dinv[:])
            idx_sb = const.tile([BLK, S // 16], mybir.dt.int16)
            nc.sync.dma_start(idx_sb[:], gidx[:])
            zt = const.tile([4, TBLW], bf16)
            nc.gpsimd.memset(zt[:], 0.0)
            nc.sync.dma_start(ag1_in[ZROW:ZROW + 4, :], zt[:])
            nc.sync.dma_start(ag2_in[ZROW:ZROW + 4, :], zt[:])

            # SBUF-resident fp32 copies of the local pre-scaled rows for the
            # elementwise self-loop term (dinv^2 * h == dinv * hs).
            hs1_all = const.tile([BLK, NBLK * F_HID], f32)
            hs2_all = const.tile([BLK, NBLK * F_HID], f32)
            nc.gpsimd.memset(hs1_all[:], 0.0)
            nc.gpsimd.memset(hs2_all[:], 0.0)

            # ---- phase 1: h_scaled = dinv * (x @ W1), locally owned rows ----
            for t in range(NBLK):
                nt = BLK if t < NBLK - 1 else LAST_BLK
                xt = xpool.tile([F_IN, BLK], bf16, tag="xt")
                nc.sync.dma_start(xt[:, :nt], xT[:, t * BLK:t * BLK + nt])
                hp = pst.tile([BLK, 512], f32, space="PSUM", tag="tmp",
                              name="hp")
                nc.tensor.matmul(out=hp[:nt, :F_HID], lhsT=xt[:, :nt], rhs=W1t[:],
                                 start=True, stop=True)
                nc.vector.tensor_scalar(
                    out=hs1_all[:nt, t * F_HID:(t + 1) * F_HID],
                    in0=hp[:nt, :F_HID],
                    scalar1=dinv[:nt, t:t + 1], scalar2=None,
                    op0=AT.mult)
                hsb = hpool.tile([BLK, TBLW], bf16, tag="hs")
                nc.scalar.activation(
                    hsb[:nt, :F_HID],
                    hs1_all[:nt, t * F_HID:(t + 1) * F_HID],
                    func=mybir.ActivationFunctionType.Copy)
                nc.sync.dma_start(ag1_in[t * BLK:t * BLK + nt, :],
                                  hsb[:nt, :])

            nc.gpsimd.collective_compute(
                "AllGather", AT.bypass,
                replica_groups=[list(range(NCORES))],
                ins=[ag1_in.opt()], outs=[ag1_out.opt()],
            )

            chunk_counter = [0]

            def run_layer(table, post_fn):
                gslot = 0
                for p in range(len(PASSES)):
                    blocks = PASSES[p]
                    pos = {b: divmod(i, 8) for i, b in enumerate(blocks)}
                    banks = {}
                    for b in blocks:
                        bank, _ = pos[b]
                        if bank not in banks:
                            banks[bank] = psb.tile([BLK, 512], f32,
                                                   space="PSUM",
                                                   name=f"bank{bank}",
                                                   tag=f"bank{bank}")
                    for w in range(NWIN):
                        groups = [(b, g) for b in blocks
                                  for g in range(int(G[b, w]))]
                        ci = 0
                        while ci < len(groups):
                            chunk = groups[ci:ci + CHUNK_GROUPS]
                            ci += len(chunk)
                            n = len(chunk) * BLK
                            mt = mpool.tile([BLK, CHUNK_GROUPS, TBLW], bf16,
                                            tag="msgs")
                            ohc = ohpool.tile([BLK, CHUNK_GROUPS * BLK], bf16,
                                              tag="oh")
                            nc.sync.dma_start(
                                ohc[:, :len(chunk) * BLK],
                                ohm[:, gslot * BLK:
                                    (gslot + len(chunk)) * BLK])
                            # split across the 4 SWDGE queues: Q7 desc-gen
                            # runs on all 4 core pairs concurrently
                            nsub = min(4, len(chunk))
                            base, rem = divmod(len(chunk), nsub)
                            j0 = 0
                            for si in range(nsub):
                                sg = base + (1 if si < rem else 0)
                                if sg == 0:
                                    continue
                                sn = sg * BLK
                                soff = gslot + j0
                                nc.gpsimd.dma_gather(
                                    out_ap=mt[:, j0:j0 + sg, :],
                                    in_ap=table[w * WIN:(w + 1) * WIN, :],
                                    idxs_ap=idx_sb[:, soff * 8:
                                                   soff * 8 + sn // 16],
                                    num_idxs=sn, num_idxs_reg=sn,
                                    elem_size=TBLW,
                                    single_packet=False,
                                    queue_num=si,
                                )
                                j0 += sg
                            for j, (b, g) in enumerate(chunk):
                                bank, off = pos[b]
                                nc.tensor.matmul(
                                    out=banks[bank][:, off * F_HID:
                                                    (off + 1) * F_HID],
                                    lhsT=ohc[:, j * BLK:(j + 1) * BLK],
                                    rhs=mt[:, j, :F_HID],
                                    start=((w, b, g) == first[(p, bank)]),
                                    stop=((w, b, g) == last[(p, bank)]),
                                    skip_group_check=True)
                                gslot += 1
                    # read each PSUM bank back whole (single reader per bank),
                    # then run the per-block post on SBUF slices
                    for bank, bt in banks.items():
                        bank_blocks = [b for b in blocks
                                       if pos[b][0] == bank]
                        nfree = len(bank_blocks) * F_HID
                        post_fn(bank, bt, bank_blocks, nfree)

            # ---- layer 1 post:
            # X = bank + dinv*hs1 (self-loop); h1s = dinv*relu(dinv*X + b1)
            # b1 == 0 fast path: dinv*relu(dinv*X) == dinv^2*relu(X).
            def post1(bank, bt, bank_blocks, nfree):
                for i, b in enumerate(bank_blocks):
                    nb = BLK if b < NBLK - 1 else LAST_BLK
                    sl = bt[:, i * F_HID:(i + 1) * F_HID]
                    hb = hs1_all[:, b * F_HID:(b + 1) * F_HID]
                    x = ppool.tile([BLK, F_HID], f32, tag="post1x", name="x")
                    nc.vector.tensor_add(out=x[:], in0=hb, in1=sl)
                    sl2 = hs2_all[:, b * F_HID:(b + 1) * F_HID]
                    if b1_nonzero:
                        h = ppool.tile([BLK, F_HID], f32, tag="post1",
                                       name="h")
                        nc.vector.tensor_scalar(out=h[:], in0=x[:],
                                                scalar1=dinv[:, b:b + 1],
                                                scalar2=None, op0=AT.mult)
                        nc.vector.tensor_add(out=h[:], in0=h[:], in1=b1t[:])
                        nc.vector.tensor_scalar(out=sl2, in0=h[:],
                                                scalar1=dinv[:, b:b + 1],
                                                scalar2=0.0, op0=AT.mult,
                                                op1=AT.max)
                    else:
                        nc.scalar.activation(
                            x[:], x[:],
                            func=mybir.ActivationFunctionType.Relu)
                        nc.vector.tensor_scalar(out=sl2, in0=x[:],
                                                scalar1=dinv2[:, b:b + 1],
                                                scalar2=None, op0=AT.mult)
                    hbf = ppool.tile([BLK, TBLW], bf16, tag="post1b",
                                     name="hbf")
                    nc.scalar.activation(
                        hbf[:, :F_HID], sl2,
                        func=mybir.ActivationFunctionType.Copy)
                    nc.sync.dma_start(ag2_in[b * BLK:b * BLK + nb, :],
                                      hbf[:nb, :])

            run_layer(ag1_out, post1)

            nc.gpsimd.collective_compute(
                "AllGather", AT.bypass,
                replica_groups=[list(range(NCORES))],
                ins=[ag2_in.opt()], outs=[ag2_out.opt()],
            )

            # ---- layer 2 post: out = dinv * ((bank + dinv*hs2) @ W2) + b2 ----
            def post2(bank, bt, bank_blocks, nfree):
                for i, b in enumerate(bank_blocks):
                    nb = BLK if b < NBLK - 1 else LAST_BLK
                    sl = bt[:, i * F_HID:(i + 1) * F_HID]
                    hb = hs2_all[:, b * F_HID:(b + 1) * F_HID]
                    ag = ppool.tile([BLK, F_HID], f32, tag="agg2", name="ag")
                    nc.vector.tensor_add(out=ag[:], in0=hb, in1=sl)
                    t2 = pst.tile([BLK, 512], f32, space="PSUM", tag="tmp",
                                  name="t2")
                    nc.tensor.transpose(
                        out=t2[0:F_HID, 0:BLK],
                        in_=ag[:],
                        identity=ident[:])
                    aT = ppool.tile([F_HID, BLK], bf16, tag="aggT", name="aT")
                    nc.scalar.activation(aT[:], t2[0:F_HID, 0:BLK],
                                         func=mybir.ActivationFunctionType.Copy)
                    nc.tensor.matmul(out=t2[:, BLK:BLK + F_OUT], lhsT=aT[:],
                                     rhs=W2t[:], start=True, stop=True)
                    o = ppool.tile([BLK, F_OUT], f32, tag="out2", name="o")
                    nc.vector.tensor_scalar(out=o[:],
                                            in0=t2[:, BLK:BLK + F_OUT],
                                            scalar1=dinv[:, b:b + 1],
                                            scalar2=None, op0=AT.mult)
                    if b2_nonzero:
                        nc.vector.tensor_add(out=o[:], in0=o[:], in1=b2t[:])
                    nc.sync.dma_start(y[b * BLK:b * BLK + nb, :], o[:nb, :])

            run_layer(ag2_out, post2)

    nc.compile()
    return nc


def _to_bf16(a):
    import ml_dtypes
    return np.asarray(a, dtype=np.float32).astype(ml_dtypes.bfloat16)


def kernel(x, W1, b1, W2, b2, edge_index):
    global LAST_RESULTS
    from concourse.bass_utils import run_bass_kernel_spmd

    x = np.asarray(x, dtype=np.float32)
    W1 = np.asarray(W1, dtype=np.float32)
    W2 = np.asarray(W2, dtype=np.float32)
    b1 = np.asarray(b1, dtype=np.float32)
    b2 = np.asarray(b2, dtype=np.float32)

    ekey = hash(np.asarray(edge_index).tobytes()) ^ hash(
        (bool(np.any(b1)), bool(np.any(b2))))
    if ekey in _CACHE:
        nc, G, S, per_core = _CACHE[ekey]
    else:
        G, S, per_core = _preprocess(edge_index)
        nc = _build(G, S, bool(np.any(b1)), bool(np.any(b2)))
        _CACHE.clear()
        _CACHE[ekey] = (nc, G, S, per_core)

    b1r = np.broadcast_to(b1, (BLK, F_HID)).copy()
    b2r = np.broadcast_to(b2, (BLK, F_OUT)).copy()
    W1b = _to_bf16(W1)
    in_maps = []
    for c in range(NCORES):
        pc = per_core[c]
        in_maps.append({
            "xT": _to_bf16(np.ascontiguousarray(x[c * NPC:(c + 1) * NPC].T)),
            "W1": W1b, "W2": _to_bf16(W2), "b1r": b1r, "b2r": b2r,
            "degT": pc["degT"], "gidx": pc["gidx"], "ohm": pc["ohm"],
        })

    res = run_bass_kernel_spmd(nc, in_maps, core_ids=list(range(NCORES)))
    LAST_RESULTS = res
    return np.concatenate([res.results[c]["y"] for c in range(NCORES)], axis=0)


# revision 12
# speedup vs baseline: 1.0145x; 1.0145x over previous
"""Two-layer GCN (BotGCN) on 8 Trainium2 NeuronCores.

Distribution: nodes partitioned contiguously across the 8 cores (12500
each). Each core owns the edges whose destination lands in its block.
Layer math is refactored so all per-edge work is a gather of pre-scaled
rows + a segment-sum:

    out[v] = dinv[v] * (sum_{e: dst=v, real} (dinv[src] * h[src])
                        + dinv[v] * h[v]) + bias

Self-loops are NOT materialized as edges: their contribution is the
elementwise term dinv[v]^2 * h[v], added from an SBUF-resident copy of
the local pre-scaled rows. This removes ~12.5K slots/core/layer and the
per-bucket self-loop concentration that inflated padding.

Per layer, per core:
  - compute locally-owned pre-scaled rows in bf16, AllGather them into a
    [8*12544, 128] bf16 table (row = 64 feats + 64 junk pad columns that
    are never read; 256B rows satisfy the gather's elem-size rule)
  - dma_gather edge messages from the table (4 windows of 2 shards so
    int16 gather indices stay in range); gather chunks rotate across the
    4 SWDGE queues so Q7 descriptor generation runs on all 4 core pairs
    concurrently (~4x)
  - segment-sum via bf16 iota-compare one-hot matrices and single-pass
    bf16 PE matmuls accumulating into fp32 PSUM, one [128, 64] tile per
    128-destination block
Layer 2 then applies W2 on the aggregate (linearity) and scales.

The edge bucketing / padding structure is computed on the host from the
integer edge list and baked into the program (the program is rebuilt per
kernel() call); it is shared by all 8 cores (max-over-cores group
counts) because every core runs the same NEFF.
"""

import numpy as np

N = 100000
NCORES = 8
NPC = N // NCORES            # 12500 nodes per core
SHARD = 12544                # 128*98: 12500 rows + zero row @12500 + pad
WIN = 2 * SHARD              # 25088 table rows per gather window (< 32768)
NWIN = 4
BLK = 128
NBLK = (NPC + BLK - 1) // BLK          # 98 destination blocks
LAST_BLK = NPC - (NBLK - 1) * BLK      # 84 nodes in the last block
F_IN, F_HID, F_OUT = 128, 64, 2
TBLW = 128                              # bf16 table row width (256B)
ZROW = NPC                              # zero row offset within a shard
# 3 passes of <=33 dst blocks: each pass holds ceil(33/8)=5 PSUM bank tiles,
# leaving banks for the transient pool (PSUM tiles pad to a full bank).
PASSES = [list(range(0, 33)), list(range(33, 66)), list(range(66, NBLK))]
CHUNK_GROUPS = 40                                      # 5120 slots per gather

_CACHE = {}

# Results of the most recent run (for the local test harness's profiling).
LAST_RESULTS = None


def _preprocess(edge_index):
    """Host-side integer bucketing of the edge list (self-loops excluded).

    Returns the shared group structure G[b, w], slot count S, and the
    per-core staged gather-index / dstrel arrays.
    """
    src = np.asarray(edge_index[0]).astype(np.int64)
    dst = np.asarray(edge_index[1]).astype(np.int64)

    # degree includes the self-loop (reference semantics)
    deg = (np.bincount(dst, minlength=N) + 1).astype(np.float32)

    core = dst // NPC
    dloc = dst % NPC
    blk = dloc // BLK
    win = src // (2 * NPC)

    cnt = np.zeros((NCORES, NBLK, NWIN), np.int64)
    np.add.at(cnt, (core, blk, win), 1)
    G = -(-cnt.max(axis=0) // BLK)        # [NBLK, NWIN] ceil, shared by cores

    # Bucket ordering: (pass, window, block) — must match the kernel loops.
    bucket_order = []
    for p in range(len(PASSES)):
        for w in range(NWIN):
            for b in PASSES[p]:
                bucket_order.append((b, w))
    nbuckets = len(bucket_order)
    ord_of = np.zeros((NBLK, NWIN), np.int64)
    sizes = np.zeros(nbuckets, np.int64)
    for i, (b, w) in enumerate(bucket_order):
        ord_of[b, w] = i
        sizes[i] = G[b, w] * BLK
    offs = np.zeros(nbuckets + 1, np.int64)
    np.cumsum(sizes, out=offs[1:])
    S = int(offs[-1])

    starts = offs[:-1]

    per_core = []
    for c in range(NCORES):
        m = core == c
        sc = src[m]
        key = ord_of[blk[m], win[m]]
        order = np.argsort(key, kind="stable")
        ks = key[order]
        bstart = np.searchsorted(ks, np.arange(nbuckets))
        rank = np.arange(len(ks)) - bstart[ks]
        slot = starts[ks] + rank

        so = sc[order]
        o = so // NPC
        wrel = (o % 2) * SHARD + (so % NPC)

        gidx = np.full(S, ZROW, np.int16)         # pad -> window zero row
        gidx[slot] = wrel.astype(np.int16)
        drel = np.full(S, -1.0, np.float32)
        drel[slot] = (dloc[m][order] % BLK).astype(np.float32)

        gidx16 = gidx.reshape(S // 16, 16).T      # [16, S/16], slot i -> [i%16, i//16]
        gidx_rep = np.tile(gidx16, (8, 1)).copy() # replicated for the 8 Q7 cores
        import ml_dtypes
        # precomputed one-hot matrices, lhsT layout: ohm[p, g*128+c] =
        # (drel[g*128+p] == c); pads (-1) give all-zero rows
        og = drel.reshape(S // BLK, BLK)
        ohm = (og[:, :, None] == np.arange(BLK, dtype=np.float32)[None, None, :])
        ohm = ohm.transpose(1, 0, 2).reshape(BLK, S).astype(ml_dtypes.bfloat16)

        degc = np.ones(NBLK * BLK, np.float32)
        degc[:NPC] = deg[c * NPC:(c + 1) * NPC]
        degT = degc.reshape(NBLK, BLK).T.copy()   # [128, NBLK]

        per_core.append({"gidx": gidx_rep, "ohm": ohm, "degT": degT})

    return G, S, per_core


def _build(G, S, b1_nonzero, b2_nonzero):
    import concourse.bacc as bacc
    import concourse.mybir as mybir
    import concourse.tile as tile
    from concourse.masks import make_identity

    f32 = mybir.dt.float32
    bf16 = mybir.dt.bfloat16
    AT = mybir.AluOpType

    # first/last matmul (w, b, g) per PSUM bank for start/stop flags.
    first, last = {}, {}
    for p in range(len(PASSES)):
        for w in range(NWIN):
            for b in PASSES[p]:
                bank = PASSES[p].index(b) // 8
                for g in range(int(G[b, w])):
                    last[(p, bank)] = (w, b, g)
                    first.setdefault((p, bank), (w, b, g))

    nc = bacc.Bacc("TRN2", target_bir_lowering=False, debug=False,
                   enable_asserts=True, num_devices=NCORES,
                   num_swdge_queues=4)
    xT = nc.dram_tensor("xT", [F_IN, NPC], bf16, kind="ExternalInput")
    W1 = nc.dram_tensor("W1", [F_IN, F_HID], bf16, kind="ExternalInput")
    W2 = nc.dram_tensor("W2", [F_HID, F_OUT], bf16, kind="ExternalInput")
    b1r = nc.dram_tensor("b1r", [BLK, F_HID], f32, kind="ExternalInput")
    b2r = nc.dram_tensor("b2r", [BLK, F_OUT], f32, kind="ExternalInput")
    degT = nc.dram_tensor("degT", [BLK, NBLK], f32, kind="ExternalInput")
    gidx = nc.dram_tensor("gidx", [BLK, S // 16], mybir.dt.int16,
                          kind="ExternalInput")
    ohm = nc.dram_tensor("ohm", [BLK, S], bf16, kind="ExternalInput")
    y = nc.dram_tensor("y", [NPC, F_OUT], f32, kind="ExternalOutput")

    with tile.TileContext(nc) as tc:
        with tc.tile_pool(name="const", bufs=1) as const, \
             tc.tile_pool(name="xt", bufs=3) as xpool, \
             tc.tile_pool(name="hs", bufs=3) as hpool, \
             tc.tile_pool(name="msgs", bufs=6) as mpool, \
             tc.tile_pool(name="oh", bufs=6) as ohpool, \
             tc.tile_pool(name="post", bufs=3) as ppool, \
             tc.tile_pool(name="psb", bufs=1, space="PSUM") as psb, \
             tc.tile_pool(name="pst", bufs=2, space="PSUM") as pst, \
             tc.tile_pool(name="dram", bufs=1, space="DRAM") as dram:

            ag1_in = dram.tile([SHARD, TBLW], bf16)
            ag1_out = dram.tile([NCORES * SHARD, TBLW], bf16,
                                addr_space="Shared")
            ag2_in = dram.tile([SHARD, TBLW], bf16)
            ag2_out = dram.tile([NCORES * SHARD, TBLW], bf16,
                                addr_space="Shared")

            # ---- constants ----
            ident = const.tile([BLK, BLK], f32)
            make_identity(nc, ident[:])
            W1t = const.tile([F_IN, F_HID], bf16)
            nc.sync.dma_start(W1t[:], W1[:])
            W2t = const.tile([F_HID, F_OUT], bf16)
            nc.sync.dma_start(W2t[:], W2[:])
            if b1_nonzero:
                b1t = const.tile([BLK, F_HID], f32)
                nc.sync.dma_start(b1t[:], b1r[:])
            if b2_nonzero:
                b2t = const.tile([BLK, F_OUT], f32)
                nc.sync.dma_start(b2t[:], b2r[:])
            degt = const.tile([BLK, NBLK], f32)
            nc.sync.dma_start(degt[:], degT[:])
            rcp = const.tile([BLK, NBLK], f32)
            nc.vector.reciprocal(rcp[:], degt[:])
            dinv = const.tile([BLK, NBLK], f32)
            nc.scalar.sqrt(dinv[:], rcp[:])
            dinv2 = const.tile([BLK, NBLK], f32)
            nc.vector.tensor_mul(dinv2[:], dinv[:], dinv[:])
            idx_sb = const.tile([BLK, S // 16], mybir.dt.int16)
            nc.sync.dma_start(idx_sb[:], gidx[:])
            zt = const.tile([4, TBLW], bf16)
            nc.gpsimd.memset(zt[:], 0.0)
            nc.sync.dma_start(ag1_in[ZROW:ZROW + 4, :], zt[:])
            nc.sync.dma_start(ag2_in[ZROW:ZROW + 4, :], zt[:])

            # SBUF-resident fp32 copies of the local pre-scaled rows for the
            # elementwise self-loop term (dinv^2 * h == dinv * hs).
            hs1_all = const.tile([BLK, NBLK * F_HID], f32)
            hs2_all = const.tile([BLK, NBLK * F_HID], f32)
            nc.gpsimd.memset(hs1_all[:], 0.0)
            nc.gpsimd.memset(hs2_all[:], 0.0)

            # ---- phase 1: h_scaled = dinv * (x @ W1), locally owned rows ----
            for t in range(NBLK):
                nt = BLK if t < NBLK - 1 else LAST_BLK
                xt = xpool.tile([F_IN, BLK], bf16, tag="xt")
                nc.sync.dma_start(xt[:, :nt], xT[:, t * BLK:t * BLK + nt])
                hp = pst.tile([BLK, 512], f32, space="PSUM", tag="tmp",
                              name="hp")
                nc.tensor.matmul(out=hp[:nt, :F_HID], lhsT=xt[:, :nt], rhs=W1t[:],
                                 start=True, stop=True)
                nc.vector.tensor_scalar(
                    out=hs1_all[:nt, t * F_HID:(t + 1) * F_HID],
                    in0=hp[:nt, :F_HID],
                    scalar1=dinv[:nt, t:t + 1], scalar2=None,
                    op0=AT.mult)
                hsb = hpool.tile([BLK, TBLW], bf16, tag="hs")
                nc.scalar.activation(
                    hsb[:nt, :F_HID],
                    hs1_all[:nt, t * F_HID:(t + 1) * F_HID],
                    func=mybir.ActivationFunctionType.Copy)
                nc.sync.dma_start(ag1_in[t * BLK:t * BLK + nt, :],
                                  hsb[:nt, :])

            nc.gpsimd.collective_compute(
                "AllGather", AT.bypass,
                replica_groups=[list(range(NCORES))],
                ins=[ag1_in.opt()], outs=[ag1_out.opt()],
            )

            chunk_counter = [0]

            def run_layer(table, post_fn):
                gslot = 0
                for p in range(len(PASSES)):
                    blocks = PASSES[p]
                    pos = {b: divmod(i, 8) for i, b in enumerate(blocks)}
                    banks = {}
                    for b in blocks:
                        bank, _ = pos[b]
                        if bank not in banks:
                            banks[bank] = psb.tile([BLK, 512], f32,
                                                   space="PSUM",
                                                   name=f"bank{bank}",
                                                   tag=f"bank{bank}")
                    for w in range(NWIN):
                        groups = [(b, g) for b in blocks
                                  for g in range(int(G[b, w]))]
                        ci = 0
                        while ci < len(groups):
                            chunk = groups[ci:ci + CHUNK_GROUPS]
                            ci += len(chunk)
                            n = len(chunk) * BLK
                            mt = mpool.tile([BLK, CHUNK_GROUPS, TBLW], bf16,
                                            tag="msgs")
                            ohc = ohpool.tile([BLK, CHUNK_GROUPS * BLK], bf16,
                                              tag="oh")
                            nc.sync.dma_start(
                                ohc[:, :len(chunk) * BLK],
                                ohm[:, gslot * BLK:
                                    (gslot + len(chunk)) * BLK])
                            # split across the 4 SWDGE queues: Q7 desc-gen
                            # runs on all 4 core pairs concurrently
                            nsub = min(4, len(chunk))
                            base, rem = divmod(len(chunk), nsub)
                            j0 = 0
                            for si in range(nsub):
                                sg = base + (1 if si < rem else 0)
                                if sg == 0:
                                    continue
                                sn = sg * BLK
                                soff = gslot + j0
                                nc.gpsimd.dma_gather(
                                    out_ap=mt[:, j0:j0 + sg, :],
                                    in_ap=table[w * WIN:(w + 1) * WIN, :],
                                    idxs_ap=idx_sb[:, soff * 8:
                                                   soff * 8 + sn // 16],
                                    num_idxs=sn, num_idxs_reg=sn,
                                    elem_size=TBLW,
                                    single_packet=False,
                                    queue_num=si,
                                )
                                j0 += sg
                            for j, (b, g) in enumerate(chunk):
                                bank, off = pos[b]
                                nc.tensor.matmul(
                                    out=banks[bank][:, off * F_HID:
                                                    (off + 1) * F_HID],
                                    lhsT=ohc[:, j * BLK:(j + 1) * BLK],
                                    rhs=mt[:, j, :F_HID],
                                    start=((w, b, g) == first[(p, bank)]),
                                    stop=((w, b, g) == last[(p, bank)]),
                                    skip_group_check=True)
                                gslot += 1
                    # read each PSUM bank back whole (single reader per bank),
                    # then run the per-block post on SBUF slices
                    for bank, bt in banks.items():
                        bank_blocks = [b for b in blocks
                                       if pos[b][0] == bank]
                        nfree = len(bank_blocks) * F_HID
                        post_fn(bank, bt, bank_blocks, nfree)

            # ---- layer 1 post:
            # X = bank + dinv*hs1 (self-loop); h1s = dinv*relu(dinv*X + b1)
            # b1 == 0 fast path: dinv*relu(dinv*X) == dinv^2*relu(X).
            def post1(bank, bt, bank_blocks, nfree):
                for i, b in enumerate(bank_blocks):
                    nb = BLK if b < NBLK - 1 else LAST_BLK
                    sl = bt[:, i * F_HID:(i + 1) * F_HID]
                    hb = hs1_all[:, b * F_HID:(b + 1) * F_HID]
                    x = ppool.tile([BLK, F_HID], f32, tag="post1x", name="x")
                    nc.vector.tensor_add(out=x[:], in0=hb, in1=sl)
                    sl2 = hs2_all[:, b * F_HID:(b + 1) * F_HID]
                    if b1_nonzero:
                        h = ppool.tile([BLK, F_HID], f32, tag="post1",
                                       name="h")
                        nc.vector.tensor_scalar(out=h[:], in0=x[:],
                                                scalar1=dinv[:, b:b + 1],
                                                scalar2=None, op0=AT.mult)
                        nc.vector.tensor_add(out=h[:], in0=h[:], in1=b1t[:])
                        nc.vector.tensor_scalar(out=sl2, in0=h[:],
                                                scalar1=dinv[:, b:b + 1],
                                                scalar2=0.0, op0=AT.mult,
                                                op1=AT.max)
                    else:
                        nc.scalar.activation(
                            x[:], x[:],
                            func=mybir.ActivationFunctionType.Relu)
                        nc.vector.tensor_scalar(out=sl2, in0=x[:],
                                                scalar1=dinv2[:, b:b + 1],
                                                scalar2=None, op0=AT.mult)
                    hbf = ppool.tile([BLK, TBLW], bf16, tag="post1b",
                                     name="hbf")
                    nc.scalar.activation(
                        hbf[:, :F_HID], sl2,
                        func=mybir.ActivationFunctionType.Copy)
                    nc.sync.dma_start(ag2_in[b * BLK:b * BLK + nb, :],
                                      hbf[:nb, :])

            run_layer(ag1_out, post1)

            nc.gpsimd.collective_compute(
                "AllGather", AT.bypass,
                replica_groups=[list(range(NCORES))],
                ins=[ag2_in.opt()], outs=[ag2_out.opt()],
            )

            # ---- layer 2 post: out = dinv * ((bank + dinv*hs2) @ W2) + b2 ----
            def post2(bank, bt, bank_blocks, nfree):
                for i, b in enumerate(bank_blocks):
                    nb = BLK if b < NBLK - 1 else LAST_BLK
                    sl = bt[:, i * F_HID:(i + 1) * F_HID]
                    hb = hs2_all[:, b * F_HID:(b + 1) * F_HID]
                    ag = ppool.tile([BLK, F_HID], f32, tag="agg2", name="ag")
                    nc.vector.tensor_add(out=ag[:], in0=hb, in1=sl)
                    t2 = pst.tile([BLK, 512], f32, space="PSUM", tag="tmp",
                                  name="t2")
                    nc.tensor.transpose(
                        out=t2[0:F_HID, 0:BLK],
                        in_=ag[:],
                        identity=ident[:])
                    aT = ppool.tile([F_HID, BLK], bf16, tag="aggT", name="aT")
                    nc.scalar.activation(aT[:], t2[0:F_HID, 0:BLK],
                                         func=mybir.ActivationFunctionType.Copy)
                    nc.tensor.matmul(out=t2[:, BLK:BLK + F_OUT], lhsT=aT[:],
                                     rhs=W2t[:], start=True, stop=True)
                    o = ppool.tile([BLK, F_OUT], f32, tag="out2", name="o")
                    nc.vector.tensor_scalar(out=o[:],
                                            in0=t2[:, BLK:BLK + F_OUT],
                                            scalar1=dinv[:, b:b + 1],
                                            scalar2=None, op0=AT.mult)
                    if b2_nonzero:
                        nc.vector.tensor_add(out=o[:], in0=o[:], in1=b2t[:])
                    nc.sync.dma_start(y[b * BLK:b * BLK + nb, :], o[:nb, :])

            run_layer(ag2_out, post2)

    nc.compile()
    return nc


def _to_bf16(a):
    import ml_dtypes
    return np.asarray(a, dtype=np.float32).astype(ml_dtypes.bfloat16)


def kernel(x, W1, b1, W2, b2, edge_index):
    global LAST_RESULTS
    from concourse.bass_utils import run_bass_kernel_spmd

    x = np.asarray(x, dtype=np.float32)
    W1 = np.asarray(W1, dtype=np.float32)
    W2 = np.asarray(W2, dtype=np.float32)
    b1 = np.asarray(b1, dtype=np.float32)
    b2 = np.asarray(b2, dtype=np.float32)

    ekey = hash(np.asarray(edge_index).tobytes()) ^ hash(
        (bool(np.any(b1)), bool(np.any(b2))))
    if ekey in _CACHE:
        nc, G, S, per_core = _CACHE[ekey]
    else:
        G, S, per_core = _preprocess(edge_index)
        nc = _build(G, S, bool(np.any(b1)), bool(np.any(b2)))
        _CACHE.clear()
        _CACHE[ekey] = (nc, G, S, per_core)

    b1r = np.broadcast_to(b1, (BLK, F_HID)).copy()
    b2r = np.broadcast_to(b2, (BLK, F_OUT)).copy()
    W1b = _to_bf16(W1)
    in_maps = []
    for c in range(NCORES):
        pc = per_core[c]
        in_maps.append({
            "xT": _to_bf16(np.ascontiguousarray(x[c * NPC:(c + 1) * NPC].T)),
            "W1": W1b, "W2": _to_bf16(W2), "b1r": b1r, "b2r": b2r,
            "degT": pc["degT"], "gidx": pc["gidx"], "ohm": pc["ohm"],
        })

    res = run_bass_kernel_spmd(nc, in_maps, core_ids=list(range(NCORES)))
    LAST_RESULTS = res
    return np.concatenate([res.results[c]["y"] for c in range(NCORES)], axis=0)


# revision 14
# speedup vs baseline: 1.0197x; 1.0051x over previous
"""Two-layer GCN (BotGCN) on 8 Trainium2 NeuronCores.

Distribution: nodes partitioned contiguously across the 8 cores (12500
each). Each core owns the edges whose destination lands in its block.
Layer math is refactored so all per-edge work is a gather of pre-scaled
rows + a segment-sum:

    out[v] = dinv[v] * (sum_{e: dst=v, real} (dinv[src] * h[src])
                        + dinv[v] * h[v]) + bias

Self-loops are NOT materialized as edges: their contribution is the
elementwise term dinv[v]^2 * h[v], added from an SBUF-resident copy of
the local pre-scaled rows. This removes ~12.5K slots/core/layer and the
per-bucket self-loop concentration that inflated padding.

Per layer, per core:
  - compute locally-owned pre-scaled rows in bf16, AllGather them into a
    [8*12544, 128] bf16 table (row = 64 feats + 64 junk pad columns that
    are never read; 256B rows satisfy the gather's elem-size rule)
  - dma_gather edge messages from the table (4 windows of 2 shards so
    int16 gather indices stay in range); gather chunks rotate across the
    4 SWDGE queues so Q7 descriptor generation runs on all 4 core pairs
    concurrently (~4x)
  - segment-sum via bf16 iota-compare one-hot matrices and single-pass
    bf16 PE matmuls accumulating into fp32 PSUM, one [128, 64] tile per
    128-destination block
Layer 2 then applies W2 on the aggregate (linearity) and scales.

The edge bucketing / padding structure is computed on the host from the
integer edge list and baked into the program (the program is rebuilt per
kernel() call); it is shared by all 8 cores (max-over-cores group
counts) because every core runs the same NEFF.
"""

import numpy as np

N = 100000
NCORES = 8
NPC = N // NCORES            # 12500 nodes per core
SHARD = 12544                # 128*98: 12500 rows + zero row @12500 + pad
WIN = 2 * SHARD              # 25088 table rows per gather window (< 32768)
NWIN = 4
BLK = 128
NBLK = (NPC + BLK - 1) // BLK          # 98 destination blocks
LAST_BLK = NPC - (NBLK - 1) * BLK      # 84 nodes in the last block
F_IN, F_HID, F_OUT = 128, 64, 2
TBLW = 128                              # bf16 table row width (256B)
ZROW = NPC                              # zero row offset within a shard
# 3 passes of <=33 dst blocks: each pass holds ceil(33/8)=5 PSUM bank tiles,
# leaving banks for the transient pool (PSUM tiles pad to a full bank).
PASSES = [list(range(0, 33)), list(range(33, 66)), list(range(66, NBLK))]
CHUNK_GROUPS = 48                                      # 6144 slots per gather

_CACHE = {}

# Results of the most recent run (for the local test harness's profiling).
LAST_RESULTS = None


def _preprocess(edge_index):
    """Host-side integer bucketing of the edge list (self-loops excluded).

    Returns the shared group structure G[b, w], slot count S, and the
    per-core staged gather-index / dstrel arrays.
    """
    src = np.asarray(edge_index[0]).astype(np.int64)
    dst = np.asarray(edge_index[1]).astype(np.int64)

    # degree includes the self-loop (reference semantics)
    deg = (np.bincount(dst, minlength=N) + 1).astype(np.float32)

    core = dst // NPC
    dloc = dst % NPC
    blk = dloc // BLK
    win = src // (2 * NPC)

    cnt = np.zeros((NCORES, NBLK, NWIN), np.int64)
    np.add.at(cnt, (core, blk, win), 1)
    G = -(-cnt.max(axis=0) // BLK)        # [NBLK, NWIN] ceil, shared by cores

    # Bucket ordering: (pass, window, block) — must match the kernel loops.
    bucket_order = []
    for p in range(len(PASSES)):
        for w in range(NWIN):
            for b in PASSES[p]:
                bucket_order.append((b, w))
    nbuckets = len(bucket_order)
    ord_of = np.zeros((NBLK, NWIN), np.int64)
    sizes = np.zeros(nbuckets, np.int64)
    for i, (b, w) in enumerate(bucket_order):
        ord_of[b, w] = i
        sizes[i] = G[b, w] * BLK
    offs = np.zeros(nbuckets + 1, np.int64)
    np.cumsum(sizes, out=offs[1:])
    S = int(offs[-1])

    starts = offs[:-1]

    per_core = []
    for c in range(NCORES):
        m = core == c
        sc = src[m]
        key = ord_of[blk[m], win[m]]
        order = np.argsort(key, kind="stable")
        ks = key[order]
        bstart = np.searchsorted(ks, np.arange(nbuckets))
        rank = np.arange(len(ks)) - bstart[ks]
        slot = starts[ks] + rank

        so = sc[order]
        o = so // NPC
        wrel = (o % 2) * SHARD + (so % NPC)

        gidx = np.full(S, ZROW, np.int16)         # pad -> window zero row
        gidx[slot] = wrel.astype(np.int16)
        drel = np.full(S, -1.0, np.float32)
        drel[slot] = (dloc[m][order] % BLK).astype(np.float32)

        gidx16 = gidx.reshape(S // 16, 16).T      # [16, S/16], slot i -> [i%16, i//16]
        gidx_rep = np.tile(gidx16, (8, 1)).copy() # replicated for the 8 Q7 cores
        import ml_dtypes
        # precomputed one-hot matrices, lhsT layout: ohm[p, g*128+c] =
        # (drel[g*128+p] == c); pads (-1) give all-zero rows
        og = drel.reshape(S // BLK, BLK)
        ohm = (og[:, :, None] == np.arange(BLK, dtype=np.float32)[None, None, :])
        ohm = ohm.transpose(1, 0, 2).reshape(BLK, S).astype(ml_dtypes.bfloat16)

        degc = np.ones(NBLK * BLK, np.float32)
        degc[:NPC] = deg[c * NPC:(c + 1) * NPC]
        degT = degc.reshape(NBLK, BLK).T.copy()   # [128, NBLK]

        per_core.append({"gidx": gidx_rep, "ohm": ohm, "degT": degT})

    return G, S, per_core


def _build(G, S, b1_nonzero, b2_nonzero):
    import concourse.bacc as bacc
    import concourse.mybir as mybir
    import concourse.tile as tile
    from concourse.masks import make_identity

    f32 = mybir.dt.float32
    bf16 = mybir.dt.bfloat16
    AT = mybir.AluOpType

    # first/last matmul (w, b, g) per PSUM bank for start/stop flags.
    first, last = {}, {}
    for p in range(len(PASSES)):
        for w in range(NWIN):
            for b in PASSES[p]:
                bank = PASSES[p].index(b) // 8
                for g in range(int(G[b, w])):
                    last[(p, bank)] = (w, b, g)
                    first.setdefault((p, bank), (w, b, g))

    nc = bacc.Bacc("TRN2", target_bir_lowering=False, debug=False,
                   enable_asserts=True, num_devices=NCORES,
                   num_swdge_queues=4)
    xT = nc.dram_tensor("xT", [F_IN, NPC], bf16, kind="ExternalInput")
    W1 = nc.dram_tensor("W1", [F_IN, F_HID], bf16, kind="ExternalInput")
    W2 = nc.dram_tensor("W2", [F_HID, F_OUT], f32, kind="ExternalInput")
    b1r = nc.dram_tensor("b1r", [BLK, F_HID], f32, kind="ExternalInput")
    b2r = nc.dram_tensor("b2r", [BLK, F_OUT], f32, kind="ExternalInput")
    degT = nc.dram_tensor("degT", [BLK, NBLK], f32, kind="ExternalInput")
    gidx = nc.dram_tensor("gidx", [BLK, S // 16], mybir.dt.int16,
                          kind="ExternalInput")
    ohm = nc.dram_tensor("ohm", [BLK, S], bf16, kind="ExternalInput")
    y = nc.dram_tensor("y", [NPC, F_OUT], f32, kind="ExternalOutput")

    with tile.TileContext(nc) as tc:
        with tc.tile_pool(name="const", bufs=1) as const, \
             tc.tile_pool(name="xt", bufs=3) as xpool, \
             tc.tile_pool(name="hs", bufs=3) as hpool, \
             tc.tile_pool(name="msgs", bufs=5) as mpool, \
             tc.tile_pool(name="oh", bufs=5) as ohpool, \
             tc.tile_pool(name="post", bufs=3) as ppool, \
             tc.tile_pool(name="psb", bufs=1, space="PSUM") as psb, \
             tc.tile_pool(name="pst", bufs=2, space="PSUM") as pst, \
             tc.tile_pool(name="dram", bufs=1, space="DRAM") as dram:

            ag1_in = dram.tile([SHARD, TBLW], bf16)
            ag1_out = dram.tile([NCORES * SHARD, TBLW], bf16,
                                addr_space="Shared")
            ag2_in = dram.tile([SHARD, TBLW], bf16)
            ag2_out = dram.tile([NCORES * SHARD, TBLW], bf16,
                                addr_space="Shared")

            # ---- constants ----
            ident = const.tile([BLK, BLK], f32)
            make_identity(nc, ident[:])
            W1t = const.tile([F_IN, F_HID], bf16)
            nc.sync.dma_start(W1t[:], W1[:])
            W2t = const.tile([F_HID, F_OUT], f32)
            nc.sync.dma_start(W2t[:], W2[:])
            if b1_nonzero:
                b1t = const.tile([BLK, F_HID], f32)
                nc.sync.dma_start(b1t[:], b1r[:])
            if b2_nonzero:
                b2t = const.tile([BLK, F_OUT], f32)
                nc.sync.dma_start(b2t[:], b2r[:])
            degt = const.tile([BLK, NBLK], f32)
            nc.sync.dma_start(degt[:], degT[:])
            rcp = const.tile([BLK, NBLK], f32)
            nc.vector.reciprocal(rcp[:], degt[:])
            dinv = const.tile([BLK, NBLK], f32)
            nc.scalar.sqrt(dinv[:], rcp[:])
            dinv2 = const.tile([BLK, NBLK], f32)
            nc.vector.tensor_mul(dinv2[:], dinv[:], dinv[:])
            idx_sb = const.tile([BLK, S // 16], mybir.dt.int16)
            nc.sync.dma_start(idx_sb[:], gidx[:])
            zt = const.tile([4, TBLW], bf16)
            nc.gpsimd.memset(zt[:], 0.0)
            nc.sync.dma_start(ag1_in[ZROW:ZROW + 4, :], zt[:])
            nc.sync.dma_start(ag2_in[ZROW:ZROW + 4, :], zt[:])

            # SBUF-resident fp32 copies of the local pre-scaled rows for the
            # elementwise self-loop term (dinv^2 * h == dinv * hs).
            hs1_all = const.tile([BLK, NBLK * F_HID], f32)
            hs2_all = const.tile([BLK, NBLK * F_HID], f32)
            nc.gpsimd.memset(hs1_all[:], 0.0)
            nc.gpsimd.memset(hs2_all[:], 0.0)

            # ---- phase 1: h_scaled = dinv * (x @ W1), locally owned rows ----
            for t in range(NBLK):
                nt = BLK if t < NBLK - 1 else LAST_BLK
                xt = xpool.tile([F_IN, BLK], bf16, tag="xt")
                nc.sync.dma_start(xt[:, :nt], xT[:, t * BLK:t * BLK + nt])
                hp = pst.tile([BLK, 512], f32, space="PSUM", tag="tmp",
                              name="hp")
                nc.tensor.matmul(out=hp[:nt, :F_HID], lhsT=xt[:, :nt], rhs=W1t[:],
                                 start=True, stop=True)
                nc.vector.tensor_scalar(
                    out=hs1_all[:nt, t * F_HID:(t + 1) * F_HID],
                    in0=hp[:nt, :F_HID],
                    scalar1=dinv[:nt, t:t + 1], scalar2=None,
                    op0=AT.mult)
                hsb = hpool.tile([BLK, TBLW], bf16, tag="hs")
                nc.scalar.activation(
                    hsb[:nt, :F_HID],
                    hs1_all[:nt, t * F_HID:(t + 1) * F_HID],
                    func=mybir.ActivationFunctionType.Copy)
                nc.sync.dma_start(ag1_in[t * BLK:t * BLK + nt, :],
                                  hsb[:nt, :])

            nc.gpsimd.collective_compute(
                "AllGather", AT.bypass,
                replica_groups=[list(range(NCORES))],
                ins=[ag1_in.opt()], outs=[ag1_out.opt()],
            )

            chunk_counter = [0]

            def run_layer(table, post_fn):
                gslot = 0
                for p in range(len(PASSES)):
                    blocks = PASSES[p]
                    pos = {b: divmod(i, 8) for i, b in enumerate(blocks)}
                    banks = {}
                    for b in blocks:
                        bank, _ = pos[b]
                        if bank not in banks:
                            banks[bank] = psb.tile([BLK, 512], f32,
                                                   space="PSUM",
                                                   name=f"bank{bank}",
                                                   tag=f"bank{bank}")
                    for w in range(NWIN):
                        groups = [(b, g) for b in blocks
                                  for g in range(int(G[b, w]))]
                        ci = 0
                        while ci < len(groups):
                            chunk = groups[ci:ci + CHUNK_GROUPS]
                            ci += len(chunk)
                            n = len(chunk) * BLK
                            mt = mpool.tile([BLK, CHUNK_GROUPS, TBLW], bf16,
                                            tag="msgs")
                            ohc = ohpool.tile([BLK, CHUNK_GROUPS * BLK], bf16,
                                              tag="oh")
                            nc.sync.dma_start(
                                ohc[:, :len(chunk) * BLK],
                                ohm[:, gslot * BLK:
                                    (gslot + len(chunk)) * BLK])
                            # split across the 4 SWDGE queues: Q7 desc-gen
                            # runs on all 4 core pairs concurrently
                            nsub = min(4, len(chunk))
                            base, rem = divmod(len(chunk), nsub)
                            j0 = 0
                            for si in range(nsub):
                                sg = base + (1 if si < rem else 0)
                                if sg == 0:
                                    continue
                                sn = sg * BLK
                                soff = gslot + j0
                                nc.gpsimd.dma_gather(
                                    out_ap=mt[:, j0:j0 + sg, :],
                                    in_ap=table[w * WIN:(w + 1) * WIN, :],
                                    idxs_ap=idx_sb[:, soff * 8:
                                                   soff * 8 + sn // 16],
                                    num_idxs=sn, num_idxs_reg=sn,
                                    elem_size=TBLW,
                                    single_packet=False,
                                    queue_num=si,
                                )
                                j0 += sg
                            for j, (b, g) in enumerate(chunk):
                                bank, off = pos[b]
                                nc.tensor.matmul(
                                    out=banks[bank][:, off * F_HID:
                                                    (off + 1) * F_HID],
                                    lhsT=ohc[:, j * BLK:(j + 1) * BLK],
                                    rhs=mt[:, j, :F_HID],
                                    start=((w, b, g) == first[(p, bank)]),
                                    stop=((w, b, g) == last[(p, bank)]),
                                    skip_group_check=True)
                                gslot += 1
                    # read each PSUM bank back whole (single reader per bank),
                    # then run the per-block post on SBUF slices
                    for bank, bt in banks.items():
                        bank_blocks = [b for b in blocks
                                       if pos[b][0] == bank]
                        nfree = len(bank_blocks) * F_HID
                        post_fn(bank, bt, bank_blocks, nfree)

            # ---- layer 1 post:
            # X = bank + dinv*hs1 (self-loop); h1s = dinv*relu(dinv*X + b1)
            # b1 == 0 fast path: dinv*relu(dinv*X) == dinv^2*relu(X).
            def post1(bank, bt, bank_blocks, nfree):
                for i, b in enumerate(bank_blocks):
                    nb = BLK if b < NBLK - 1 else LAST_BLK
                    sl = bt[:, i * F_HID:(i + 1) * F_HID]
                    hb = hs1_all[:, b * F_HID:(b + 1) * F_HID]
                    x = ppool.tile([BLK, F_HID], f32, tag="post1x", name="x")
                    nc.vector.tensor_add(out=x[:], in0=hb, in1=sl)
                    sl2 = hs2_all[:, b * F_HID:(b + 1) * F_HID]
                    if b1_nonzero:
                        h = ppool.tile([BLK, F_HID], f32, tag="post1",
                                       name="h")
                        nc.vector.tensor_scalar(out=h[:], in0=x[:],
                                                scalar1=dinv[:, b:b + 1],
                                                scalar2=None, op0=AT.mult)
                        nc.vector.tensor_add(out=h[:], in0=h[:], in1=b1t[:])
                        nc.vector.tensor_scalar(out=sl2, in0=h[:],
                                                scalar1=dinv[:, b:b + 1],
                                                scalar2=0.0, op0=AT.mult,
                                                op1=AT.max)
                    else:
                        nc.scalar.activation(
                            x[:], x[:],
                            func=mybir.ActivationFunctionType.Relu)
                        nc.vector.tensor_scalar(out=sl2, in0=x[:],
                                                scalar1=dinv2[:, b:b + 1],
                                                scalar2=None, op0=AT.mult)
                    hbf = ppool.tile([BLK, TBLW], bf16, tag="post1b",
                                     name="hbf")
                    nc.scalar.activation(
                        hbf[:, :F_HID], sl2,
                        func=mybir.ActivationFunctionType.Copy)
                    nc.sync.dma_start(ag2_in[b * BLK:b * BLK + nb, :],
                                      hbf[:nb, :])

            run_layer(ag1_out, post1)

            nc.gpsimd.collective_compute(
                "AllGather", AT.bypass,
                replica_groups=[list(range(NCORES))],
                ins=[ag2_in.opt()], outs=[ag2_out.opt()],
            )

            # ---- layer 2 post: out = dinv * ((bank + dinv*hs2) @ W2) + b2 ----
            def post2(bank, bt, bank_blocks, nfree):
                for i, b in enumerate(bank_blocks):
                    nb = BLK if b < NBLK - 1 else LAST_BLK
                    sl = bt[:, i * F_HID:(i + 1) * F_HID]
                    hb = hs2_all[:, b * F_HID:(b + 1) * F_HID]
                    ag = ppool.tile([BLK, F_HID], f32, tag="agg2", name="ag")
                    nc.vector.tensor_add(out=ag[:], in0=hb, in1=sl)
                    t2 = pst.tile([BLK, 512], f32, space="PSUM", tag="tmp",
                                  name="t2")
                    nc.tensor.transpose(
                        out=t2[0:F_HID, 0:BLK],
                        in_=ag[:],
                        identity=ident[:])
                    aT = ppool.tile([F_HID, BLK], f32, tag="aggT", name="aT")
                    nc.scalar.activation(aT[:], t2[0:F_HID, 0:BLK],
                                         func=mybir.ActivationFunctionType.Copy)
                    nc.tensor.matmul(out=t2[:, BLK:BLK + F_OUT], lhsT=aT[:],
                                     rhs=W2t[:], start=True, stop=True)
                    o = ppool.tile([BLK, F_OUT], f32, tag="out2", name="o")
                    nc.vector.tensor_scalar(out=o[:],
                                            in0=t2[:, BLK:BLK + F_OUT],
                                            scalar1=dinv[:, b:b + 1],
                                            scalar2=None, op0=AT.mult)
                    if b2_nonzero:
                        nc.vector.tensor_add(out=o[:], in0=o[:], in1=b2t[:])
                    nc.sync.dma_start(y[b * BLK:b * BLK + nb, :], o[:nb, :])

            run_layer(ag2_out, post2)

    nc.compile()
    return nc


def _to_bf16(a):
    import ml_dtypes
    return np.asarray(a, dtype=np.float32).astype(ml_dtypes.bfloat16)


def kernel(x, W1, b1, W2, b2, edge_index):
    global LAST_RESULTS
    from concourse.bass_utils import run_bass_kernel_spmd

    x = np.asarray(x, dtype=np.float32)
    W1 = np.asarray(W1, dtype=np.float32)
    W2 = np.asarray(W2, dtype=np.float32)
    b1 = np.asarray(b1, dtype=np.float32)
    b2 = np.asarray(b2, dtype=np.float32)

    ekey = hash(np.asarray(edge_index).tobytes()) ^ hash(
        (bool(np.any(b1)), bool(np.any(b2))))
    if ekey in _CACHE:
        nc, G, S, per_core = _CACHE[ekey]
    else:
        G, S, per_core = _preprocess(edge_index)
        nc = _build(G, S, bool(np.any(b1)), bool(np.any(b2)))
        _CACHE.clear()
        _CACHE[ekey] = (nc, G, S, per_core)

    b1r = np.broadcast_to(b1, (BLK, F_HID)).copy()
    b2r = np.broadcast_to(b2, (BLK, F_OUT)).copy()
    W1b = _to_bf16(W1)
    in_maps = []
    for c in range(NCORES):
        pc = per_core[c]
        in_maps.append({
            "xT": _to_bf16(np.ascontiguousarray(x[c * NPC:(c + 1) * NPC].T)),
            "W1": W1b, "W2": W2, "b1r": b1r, "b2r": b2r,
            "degT": pc["degT"], "gidx": pc["gidx"], "ohm": pc["ohm"],
        })

    res = run_bass_kernel_spmd(nc, in_maps, core_ids=list(range(NCORES)))
    LAST_RESULTS = res
    return np.concatenate([res.results[c]["y"] for c in range(NCORES)], axis=0)
